# revision 3
# baseline (speedup 1.0000x reference)
import os
import sys
import tempfile

sys.path.insert(0, "/opt/trn_rl_repo")

import numpy as np
import ml_dtypes

import concourse.bacc as bacc
import concourse.mybir as mybir
import concourse.tile as tile
from concourse.bass_utils import run_bass_kernel_spmd

f32 = mybir.dt.float32
f32r = mybir.dt.float32r
bf16 = mybir.dt.bfloat16
AF = mybir.ActivationFunctionType
ALU = mybir.AluOpType
AX = mybir.AxisListType

# Problem dims (hardcoded per contract)
R, B, F, C, NCLS = 32, 4096, 256, 4, 1000
KK, PAD = 5, 1
L0, L1 = 254, 127          # conv1 out, pool1 out
J2 = 62                    # pool2 out positions
NCORE = 8
BL = B // NCORE            # 512 batch per core
NH = NCLS // 2             # 500, free-dim tile of expert matmul

# conv2 j2-blocks
SZ = [13, 13, 13, 13, 10]
JB0 = [0, 13, 26, 39, 52]                    # j2 block starts
BAND = []                                    # l1 band per block
for jb in range(5):
    lo = max(0, 26 * jb - 1)
    hi = min(126, 26 * jb + 2 * SZ[jb] + 2)
    BAND.append((lo, hi - lo + 1))
KJB = [4 * n for _, n in BAND]               # [116,120,120,120,96]
MJB = [8 * s for s in SZ]                    # [104,104,104,104,80]
W1COLS = [4 * n for _, n in BAND for _ in (0, 1)]  # per (jb,e) tile


def _conv1_np(x, w):
    # x: [N, F], w: [C,1,KK] -> [N, C, L0] with pad=1
    xp = np.pad(x, ((0, 0), (PAD, PAD)))
    out = np.zeros((x.shape[0], C, L0), np.float32)
    for c in range(C):
        for k in range(KK):
            out[:, c, :] += w[c, 0, k] * xp[:, k:k + L0]
    return out


def _build_host(proto, c1w, c1b, c2w, c2b, fc1w, fc1b, fc2w):
    bf = ml_dtypes.bfloat16
    # W1: dense conv1 matrix [F, sum(W1COLS)] in (jb,e) tile column order,
    # within tile col = l1loc*4 + c, conv output position (c, l0=2*l1+e)
    tot = sum(W1COLS)
    W1 = np.zeros((F, tot), np.float32)
    off = 0
    colmeta = []  # (jb, e, band_start, ncols)
    for jb in range(5):
        b0, bl = BAND[jb]
        for e in (0, 1):
            for l1loc in range(bl):
                l0 = 2 * (b0 + l1loc) + e
                for c in range(C):
                    col = off + l1loc * 4 + c
                    for k in range(KK):
                        f = l0 + k - 1
                        if 0 <= f < F:
                            W1[f, col] += c1w[c, 0, k]
            colmeta.append((jb, e, b0, 4 * bl))
            off += 4 * bl
    # Q: per-partition scalars [128, R*10] f32; col = r*10 + (jb*2+e)
    c1p = _conv1_np(proto, c1w)  # [R, C, L0]
    Q = np.zeros((128, R * 10), np.float32)
    for r in range(R):
        t = 0
        for jb in range(5):
            b0, bl = BAND[jb]
            for e in (0, 1):
                for l1loc in range(bl):
                    l0 = 2 * (b0 + l1loc) + e
                    for c in range(C):
                        Q[l1loc * 4 + c, r * 10 + t] = c1b[c] - c1p[r, c, l0]
                t += 1
    # W2B: banded conv2 [128, 5*128] bf16; block jb at free offset jb*128,
    # rows (l1loc, ci), cols (e2, j2loc, co); includes 0.5 pool1 scale
    W2B = np.zeros((128, 5 * 128), np.float32)
    for jb in range(5):
        b0, bl = BAND[jb]
        for e2 in (0, 1):
            for j2loc in range(SZ[jb]):
                l2 = 26 * jb + 2 * j2loc + e2
                for co in range(C):
                    col = e2 * 4 * SZ[jb] + j2loc * 4 + co
                    for kk in range(KK):
                        l1 = l2 - 1 + kk
                        if b0 <= l1 < b0 + bl:
                            for ci in range(C):
                                W2B[(l1 - b0) * 4 + ci, jb * 128 + col] += (
                                    0.5 * c2w[co, ci, kk])
    # B2V: relu2 bias [128, 5]
    B2V = np.zeros((128, 5), np.float32)
    for jb in range(5):
        for e2 in (0, 1):
            for j2loc in range(SZ[jb]):
                for co in range(C):
                    B2V[e2 * 4 * SZ[jb] + j2loc * 4 + co, jb] = c2b[co]
    # FC1W: [128, 5*124] bf16; block jb rows (e2,j2loc,co) -> 0.5*fc1w[co*62+j2]
    FC1W = np.zeros((128, 5 * 124), np.float32)
    for jb in range(5):
        for e2 in (0, 1):
            for j2loc in range(SZ[jb]):
                j2 = JB0[jb] + j2loc
                for co in range(C):
                    FC1W[e2 * 4 * SZ[jb] + j2loc * 4 + co,
                         jb * 124:(jb + 1) * 124] = 0.5 * fc1w[co * 62 + j2, :]
    FC1B = np.zeros((128, 1), np.float32)
    FC1B[:124, 0] = fc1b
    FC2W = np.zeros((128, 1), np.float32)
    FC2W[:124, 0] = fc2w[:, 0]
    IDT = np.eye(32, dtype=bf)
    return (W1.astype(bf), Q, W2B.astype(bf), B2V, FC1W.astype(bf), FC1B,
            FC2W.astype(bf), IDT)


def _build_program():
    nc = bacc.Bacc("TRN2", target_bir_lowering=False, debug=False,
                   num_devices=NCORE)
    TOT1 = sum(W1COLS)
    dT_e = nc.declare_dram_parameter("dT", [F, BL], f32, isOutput=False)
    dTb_e = nc.declare_dram_parameter("dTb", [F, BL], bf16, isOutput=False)
    W1_e = nc.declare_dram_parameter("W1", [F, TOT1], bf16, isOutput=False)
    Q_e = nc.declare_dram_parameter("Q", [128, R * 10], f32, isOutput=False)
    W2B_e = nc.declare_dram_parameter("W2B", [128, 5 * 128], bf16, isOutput=False)
    B2V_e = nc.declare_dram_parameter("B2V", [128, 5], f32, isOutput=False)
    FC1W_e = nc.declare_dram_parameter("FC1W", [128, 5 * 124], bf16, isOutput=False)
    FC1B_e = nc.declare_dram_parameter("FC1B", [128, 1], f32, isOutput=False)
    FC2W_e = nc.declare_dram_parameter("FC2W", [128, 1], bf16, isOutput=False)
    FC2B_e = nc.declare_dram_parameter("FC2B", [1, 1], f32, isOutput=False)
    IDT_e = nc.declare_dram_parameter("IDT", [32, 32], bf16, isOutput=False)
    CB_e = nc.declare_dram_parameter("CB", [R, NCLS], f32r, isOutput=False)
    CW_e = nc.declare_dram_parameter("CW", [R, F, NCLS], f32r, isOutput=False)
    OUT_e = nc.declare_dram_parameter("OUT", [BL, NCLS], f32, isOutput=True)

    # tile column offsets of W1 per (jb,e)
    w1off = np.cumsum([0] + W1COLS[:-1])

    with tile.TileContext(nc) as tc:
        with (
            tc.tile_pool(name="const", bufs=1) as cp,
            tc.tile_pool(name="work", bufs=1) as wp,
        ):
            dT = [cp.tile([128, BL], f32, tag=f"dT{k}", name=f"dT{k}") for k in range(2)]
            dTb = [cp.tile([128, BL], bf16, tag=f"dTb{k}", name=f"dTb{k}") for k in range(2)]
            W1 = [cp.tile([128, TOT1], bf16, tag=f"W1{k}", name=f"W1t{k}") for k in range(2)]
            Qs = cp.tile([128, R * 10], f32, tag="Qs")
            W2B = cp.tile([128, 5 * 128], bf16, tag="W2B")
            B2V = cp.tile([128, 5], f32, tag="B2V")
            FC1W = cp.tile([128, 5 * 124], bf16, tag="FC1W")
            FC1B = cp.tile([128, 1], f32, tag="FC1B")
            FC2W = cp.tile([128, 1], bf16, tag="FC2W")
            FC2B = cp.tile([1, 1], f32, tag="FC2B")
            IDT = cp.tile([32, 32], bf16, tag="IDT")
            CBs = cp.tile([R, NCLS], f32r, tag="CBs")
            fsi = cp.tile([R, BL], bf16, tag="fsi")
            eRows = cp.tile([1, R * BL], f32, tag="eRows")
            eTr = cp.tile([R, BL], f32r, tag="eTr")
            recip = cp.tile([128, 4], f32, tag="recip")
            c1d = [cp.tile([128, BL], bf16, tag=f"c1d{t}", name=f"c1d{t}") for t in range(10)]

            for k in range(2):
                nc.sync.dma_start(dT[k][:], dT_e[k * 128:(k + 1) * 128, :])
                nc.sync.dma_start(dTb[k][:], dTb_e[k * 128:(k + 1) * 128, :])
                nc.sync.dma_start(W1[k][:], W1_e[k * 128:(k + 1) * 128, :])
            nc.sync.dma_start(Qs[:], Q_e[:])
            nc.sync.dma_start(W2B[:], W2B_e[:])
            nc.sync.dma_start(B2V[:], B2V_e[:])
            nc.sync.dma_start(FC1W[:], FC1W_e[:])
            nc.sync.dma_start(FC1B[:], FC1B_e[:])
            nc.sync.dma_start(FC2W[:], FC2W_e[:])
            nc.sync.dma_start(FC2B[:], FC2B_e[:])
            nc.sync.dma_start(IDT[:], IDT_e[:])
            nc.sync.dma_start(CBs[:], CB_e[:])

            with tc.tile_pool(name="gps", bufs=1, space="PSUM") as gps:
                # ---- G1: conv1-dense matmuls -> c1d tiles (bf16) ----
                for t in range(10):
                    ncol = W1COLS[t]
                    off = int(w1off[t])
                    pg = gps.tile([128, BL], f32, tag="psg", name=f"psg{t}")
                    for k in range(2):
                        nc.tensor.matmul(
                            pg[0:ncol, :], W1[k][:, off:off + ncol], dTb[k][:],
                            start=(k == 0), stop=(k == 1))
                    nc.scalar.activation(c1d[t][0:ncol, :], pg[0:ncol, :],
                                         AF.Copy, bias=0.0, scale=1.0)

                # ---- G2: per-rule gating ----
                for r in range(R):
                    zpre = []
                    psz = gps.tile([128, BL], f32, tag="psz", name=f"psz{r}")
                    for jb in range(5):
                        kj, mj = KJB[jb], MJB[jb]
                        rl0 = wp.tile([128, BL], bf16, tag="rl0", name=f"rl0_{r}_{jb}")
                        rl1 = wp.tile([128, BL], bf16, tag="rl1", name=f"rl1_{r}_{jb}")
                        nc.vector.tensor_scalar(
                            rl0[0:kj, :], c1d[2 * jb][0:kj, :],
                            Qs[0:kj, r * 10 + 2 * jb:r * 10 + 2 * jb + 1],
                            0.0, ALU.add, ALU.max)
                        nc.vector.tensor_scalar(
                            rl1[0:kj, :], c1d[2 * jb + 1][0:kj, :],
                            Qs[0:kj, r * 10 + 2 * jb + 1:r * 10 + 2 * jb + 2],
                            0.0, ALU.add, ALU.max)
                        h1b = wp.tile([128, BL], bf16, tag="h1b", name=f"h1b_{r}_{jb}")
                        nc.vector.tensor_tensor(
                            h1b[0:kj, :], rl0[0:kj, :], rl1[0:kj, :], ALU.add)
                        ps2 = gps.tile([128, BL], f32, tag="ps2", name=f"ps2_{r}_{jb}")
                        nc.tensor.matmul(
                            ps2[0:mj, :],
                            W2B[0:kj, jb * 128:jb * 128 + mj],
                            h1b[0:kj, :], start=True, stop=True)
                        zp = wp.tile([128, BL], bf16, tag=f"zpre{jb}", name=f"zp_{r}_{jb}")
                        nc.scalar.activation(zp[0:mj, :], ps2[0:mj, :],
                                             AF.Relu, bias=B2V[0:mj, jb:jb + 1],
                                             scale=1.0)
                        zpre.append(zp)
                        nc.tensor.matmul(
                            psz[0:124, :],
                            FC1W[0:mj, jb * 124:(jb + 1) * 124],
                            zp[0:mj, :], start=(jb == 0), stop=(jb == 4))
                    zb = wp.tile([128, BL], bf16, tag="zb", name=f"zb{r}")
                    nc.scalar.activation(zb[0:124, :], psz[0:124, :],
                                         AF.Relu, bias=FC1B[0:124, :], scale=1.0)
                    psf = gps.tile([1, BL], f32, tag="psf", name=f"psf{r}")
                    nc.tensor.matmul(psf[0:1, :], FC2W[0:124, 0:1],
                                     zb[0:124, :], start=True, stop=True)
                    fst = wp.tile([1, BL], bf16, tag="fst", name=f"fst{r}")
                    nc.scalar.activation(fst[:], psf[0:1, :], AF.Tanh,
                                         bias=FC2B[0:1, :], scale=1.0)
                    nc.sync.dma_start(fsi[r:r + 1, :], fst[:])
                    nc.scalar.activation(eRows[0:1, r * BL:(r + 1) * BL],
                                         fst[:], AF.Exp, bias=0.0, scale=1.0)

                # ---- softmax pieces ----
                nc.scalar.activation(eTr[:], fsi[:], AF.Exp, bias=0.0, scale=1.0)
                for m in range(4):
                    pst = gps.tile([128, 32], bf16, tag="pst", name=f"pst{m}")
                    nc.tensor.transpose(pst[:], fsi[:, m * 128:(m + 1) * 128],
                                        IDT[:])
                    fse = wp.tile([128, 32], f32, tag="fse", name=f"fse{m}")
                    nc.scalar.activation(fse[:], pst[:], AF.Exp, bias=0.0,
                                         scale=1.0)
                    ssum = wp.tile([128, 1], f32, tag="ssum", name=f"ssum{m}")
                    nc.vector.reduce_sum(ssum[:], fse[:], AX.X)
                    nc.vector.reciprocal(recip[:, m:m + 1], ssum[:])

            # ---- expert phase ----
            with tc.tile_pool(name="eps", bufs=1, space="PSUM") as epp:
                eps = [epp.tile([128, NH], f32, tag=f"eps{g}", name=f"eps{g}") for g in range(8)]
                for r in range(R):
                    ebc = wp.tile([128, BL], f32, tag="ebc", name=f"ebc{r}")
                    nc.gpsimd.partition_broadcast(ebc[:], eRows[0:1, r * BL:(r + 1) * BL])
                    sd = []
                    for k in range(2):
                        sdk = wp.tile([128, BL], f32r, tag=f"sd{k}", name=f"sd_{r}_{k}")
                        nc.vector.tensor_tensor(sdk[:], dT[k][:], ebc[:],
                                                ALU.mult)
                        sd.append(sdk)
                    for n in range(2):
                        for k in range(2):
                            wt = wp.tile([128, NH], f32r, tag="wt", name=f"wt_{r}_{n}_{k}")
                            nc.sync.dma_start(
                                wt[:],
                                CW_e[r, k * 128:(k + 1) * 128,
                                     n * NH:(n + 1) * NH])
                            for m in range(4):
                                nc.tensor.matmul(
                                    eps[n * 4 + m][:],
                                    sd[k][:, m * 128:(m + 1) * 128], wt[:],
                                    start=(r == 0 and k == 0), stop=False)
                # bias matmul + drain
                for n in range(2):
                    for m in range(4):
                        g = n * 4 + m
                        nc.tensor.matmul(
                            eps[g][:], eTr[:, m * 128:(m + 1) * 128],
                            CBs[:, n * NH:(n + 1) * NH],
                            start=False, stop=True)
                        osb = wp.tile([128, NH], f32, tag="osb", name=f"osb_{n}_{m}")
                        nc.scalar.activation(osb[:], eps[g][:], AF.Copy,
                                             bias=0.0, scale=recip[:, m:m + 1])
                        nc.sync.dma_start(
                            OUT_e[m * 128:(m + 1) * 128, n * NH:(n + 1) * NH],
                            osb[:])
    nc.compile()
    return nc


_CACHE = {}


def kernel(data, proto, conv1_w, conv1_b, conv2_w, conv2_b,
           fc1_w, fc1_b, fc2_w, fc2_b, consq_w, consq_b, is_train=0,
           trace=False, tmpdir=None):
    bf = ml_dtypes.bfloat16
    data = np.asarray(data, np.float32)
    (W1, Q, W2B, B2V, FC1W, FC1B, FC2W, IDT) = _build_host(
        np.asarray(proto, np.float32), np.asarray(conv1_w, np.float32),
        np.asarray(conv1_b, np.float32), np.asarray(conv2_w, np.float32),
        np.asarray(conv2_b, np.float32), np.asarray(fc1_w, np.float32),
        np.asarray(fc1_b, np.float32), np.asarray(fc2_w, np.float32))
    if "nc" not in _CACHE:
        _CACHE["nc"] = _build_program()
    nc = _CACHE["nc"]

    CW = np.ascontiguousarray(np.asarray(consq_w, np.float32))
    CB = np.ascontiguousarray(np.asarray(consq_b, np.float32))
    FC2B = np.array([[np.asarray(fc2_b, np.float32).reshape(-1)[0]]], np.float32)
    shared = dict(W1=np.ascontiguousarray(W1), Q=np.ascontiguousarray(Q),
                  W2B=np.ascontiguousarray(W2B), B2V=np.ascontiguousarray(B2V),
                  FC1W=np.ascontiguousarray(FC1W),
                  FC1B=np.ascontiguousarray(FC1B),
                  FC2W=np.ascontiguousarray(FC2W), FC2B=FC2B,
                  IDT=np.ascontiguousarray(IDT), CB=CB, CW=CW)
    in_maps = []
    for i in range(NCORE):
        dsl = data[i * BL:(i + 1) * BL, :]
        dTi = np.ascontiguousarray(dsl.T)
        in_maps.append(dict(shared, dT=dTi,
                            dTb=np.ascontiguousarray(dTi.astype(bf))))
    res = run_bass_kernel_spmd(
        nc, in_maps, list(range(NCORE)), trace=trace,
        tmpdir=tmpdir or (tempfile.mkdtemp(prefix="moek_") if trace else None))
    out = np.concatenate([res.results[i]["OUT"] for i in range(NCORE)], axis=0)
    kernel.last_exec_time_ns = res.exec_time_ns
    return out


# revision 5
# speedup vs baseline: 1.3205x; 1.3205x over previous
import os
import sys
import tempfile

sys.path.insert(0, "/opt/trn_rl_repo")

import numpy as np
import ml_dtypes

import concourse.bacc as bacc
import concourse.mybir as mybir
import concourse.tile as tile
from concourse.bass_utils import run_bass_kernel_spmd

f32 = mybir.dt.float32
f32r = mybir.dt.float32r
bf16 = mybir.dt.bfloat16
AF = mybir.ActivationFunctionType
ALU = mybir.AluOpType
AX = mybir.AxisListType

# Problem dims (hardcoded per contract)
R, B, F, C, NCLS = 32, 4096, 256, 4, 1000
KK, PAD = 5, 1
L0, L1 = 254, 127          # conv1 out, pool1 out
J2 = 62                    # pool2 out positions
NCORE = 8
BL = B // NCORE            # 512 batch per core
NH = NCLS // 2             # 500, free-dim tile of expert matmul

# conv2 j2-blocks
SZ = [13, 13, 13, 13, 10]
JB0 = [0, 13, 26, 39, 52]                    # j2 block starts
BAND = []                                    # l1 band per block
for jb in range(5):
    lo = max(0, 26 * jb - 1)
    hi = min(126, 26 * jb + 2 * SZ[jb] + 2)
    BAND.append((lo, hi - lo + 1))
KJB = [4 * n for _, n in BAND]               # [116,120,120,120,96]
MJB = [8 * s for s in SZ]                    # [104,104,104,104,80]
W1COLS = [4 * n for _, n in BAND for _ in (0, 1)]  # per (jb,e) tile


def _conv1_np(x, w):
    # x: [N, F], w: [C,1,KK] -> [N, C, L0] with pad=1
    xp = np.pad(x, ((0, 0), (PAD, PAD)))
    out = np.zeros((x.shape[0], C, L0), np.float32)
    for c in range(C):
        for k in range(KK):
            out[:, c, :] += w[c, 0, k] * xp[:, k:k + L0]
    return out


def _build_host(proto, c1w, c1b, c2w, c2b, fc1w, fc1b, fc2w):
    bf = ml_dtypes.bfloat16
    # W1: dense conv1 matrix [F, sum(W1COLS)] in (jb,e) tile column order,
    # within tile col = l1loc*4 + c, conv output position (c, l0=2*l1+e)
    tot = sum(W1COLS)
    W1 = np.zeros((F, tot), np.float32)
    off = 0
    colmeta = []  # (jb, e, band_start, ncols)
    for jb in range(5):
        b0, bl = BAND[jb]
        for e in (0, 1):
            for l1loc in range(bl):
                l0 = 2 * (b0 + l1loc) + e
                for c in range(C):
                    col = off + l1loc * 4 + c
                    for k in range(KK):
                        f = l0 + k - 1
                        if 0 <= f < F:
                            W1[f, col] += c1w[c, 0, k]
            colmeta.append((jb, e, b0, 4 * bl))
            off += 4 * bl
    # Q: per-partition scalars [128, R*10] f32; col = r*10 + (jb*2+e)
    c1p = _conv1_np(proto, c1w)  # [R, C, L0]
    Q = np.zeros((128, R * 10), np.float32)
    for r in range(R):
        t = 0
        for jb in range(5):
            b0, bl = BAND[jb]
            for e in (0, 1):
                for l1loc in range(bl):
                    l0 = 2 * (b0 + l1loc) + e
                    for c in range(C):
                        Q[l1loc * 4 + c, r * 10 + t] = c1b[c] - c1p[r, c, l0]
                t += 1
    # W2B: banded conv2 [128, 5*128] bf16; block jb at free offset jb*128,
    # rows (l1loc, ci), cols (e2, j2loc, co); includes 0.5 pool1 scale
    W2B = np.zeros((128, 5 * 128), np.float32)
    for jb in range(5):
        b0, bl = BAND[jb]
        for e2 in (0, 1):
            for j2loc in range(SZ[jb]):
                l2 = 26 * jb + 2 * j2loc + e2
                for co in range(C):
                    col = e2 * 4 * SZ[jb] + j2loc * 4 + co
                    for kk in range(KK):
                        l1 = l2 - 1 + kk
                        if b0 <= l1 < b0 + bl:
                            for ci in range(C):
                                W2B[(l1 - b0) * 4 + ci, jb * 128 + col] += (
                                    0.5 * c2w[co, ci, kk])
    # B2V: relu2 bias [128, 5]
    B2V = np.zeros((128, 5), np.float32)
    for jb in range(5):
        for e2 in (0, 1):
            for j2loc in range(SZ[jb]):
                for co in range(C):
                    B2V[e2 * 4 * SZ[jb] + j2loc * 4 + co, jb] = c2b[co]
    # FC1W: [128, 5*124] bf16; block jb rows (e2,j2loc,co) -> 0.5*fc1w[co*62+j2]
    FC1W = np.zeros((128, 5 * 124), np.float32)
    for jb in range(5):
        for e2 in (0, 1):
            for j2loc in range(SZ[jb]):
                j2 = JB0[jb] + j2loc
                for co in range(C):
                    FC1W[e2 * 4 * SZ[jb] + j2loc * 4 + co,
                         jb * 124:(jb + 1) * 124] = 0.5 * fc1w[co * 62 + j2, :]
    FC1B = np.zeros((128, 1), np.float32)
    FC1B[:124, 0] = fc1b
    FC2W = np.zeros((128, 1), np.float32)
    FC2W[:124, 0] = fc2w[:, 0]
    IDT = np.eye(32, dtype=bf)
    return (W1.astype(bf), Q, W2B.astype(bf), B2V, FC1W.astype(bf), FC1B,
            FC2W.astype(bf), IDT)


def _build_program():
    nc = bacc.Bacc("TRN2", target_bir_lowering=False, debug=False,
                   num_devices=NCORE)
    TOT1 = sum(W1COLS)
    dT_e = nc.declare_dram_parameter("dT", [F, BL], f32, isOutput=False)
    dTb_e = nc.declare_dram_parameter("dTb", [F, BL], bf16, isOutput=False)
    W1_e = nc.declare_dram_parameter("W1", [F, TOT1], bf16, isOutput=False)
    Q_e = nc.declare_dram_parameter("Q", [128, R * 10], f32, isOutput=False)
    W2B_e = nc.declare_dram_parameter("W2B", [128, 5 * 128], bf16, isOutput=False)
    B2V_e = nc.declare_dram_parameter("B2V", [128, 5], f32, isOutput=False)
    FC1W_e = nc.declare_dram_parameter("FC1W", [128, 5 * 124], bf16, isOutput=False)
    FC1B_e = nc.declare_dram_parameter("FC1B", [128, 1], f32, isOutput=False)
    FC2W_e = nc.declare_dram_parameter("FC2W", [128, 1], bf16, isOutput=False)
    FC2B_e = nc.declare_dram_parameter("FC2B", [1, 1], f32, isOutput=False)
    IDT_e = nc.declare_dram_parameter("IDT", [32, 32], bf16, isOutput=False)
    CB_e = nc.declare_dram_parameter("CB", [R, NCLS], bf16, isOutput=False)
    CW_e = nc.declare_dram_parameter("CW", [R, F, NCLS], bf16, isOutput=False)
    OUT_e = nc.declare_dram_parameter("OUT", [BL, NCLS], f32, isOutput=True)

    # tile column offsets of W1 per (jb,e)
    w1off = np.cumsum([0] + W1COLS[:-1])

    with tile.TileContext(nc) as tc:
        with (
            tc.tile_pool(name="const", bufs=1) as cp,
            tc.tile_pool(name="work", bufs=1) as wp,
        ):
            dT = [cp.tile([128, BL], f32, tag=f"dT{k}", name=f"dT{k}") for k in range(2)]
            dTb = [cp.tile([128, BL], bf16, tag=f"dTb{k}", name=f"dTb{k}") for k in range(2)]
            W1 = [cp.tile([128, TOT1], bf16, tag=f"W1{k}", name=f"W1t{k}") for k in range(2)]
            Qs = cp.tile([128, R * 10], f32, tag="Qs")
            W2B = cp.tile([128, 5 * 128], bf16, tag="W2B")
            B2V = cp.tile([128, 5], f32, tag="B2V")
            FC1W = cp.tile([128, 5 * 124], bf16, tag="FC1W")
            FC1B = cp.tile([128, 1], f32, tag="FC1B")
            FC2W = cp.tile([128, 1], bf16, tag="FC2W")
            FC2B = cp.tile([1, 1], f32, tag="FC2B")
            IDT = cp.tile([32, 32], bf16, tag="IDT")
            CBs = cp.tile([R, NCLS], bf16, tag="CBs")
            fsi = cp.tile([R, BL], bf16, tag="fsi")
            eRows = cp.tile([1, R * BL], f32, tag="eRows")
            eTr = cp.tile([R, BL], bf16, tag="eTr")
            recip = cp.tile([128, 4], f32, tag="recip")
            c1d = [cp.tile([128, BL], bf16, tag=f"c1d{t}", name=f"c1d{t}") for t in range(10)]

            for k in range(2):
                nc.sync.dma_start(dT[k][:], dT_e[k * 128:(k + 1) * 128, :])
                nc.sync.dma_start(dTb[k][:], dTb_e[k * 128:(k + 1) * 128, :])
                nc.sync.dma_start(W1[k][:], W1_e[k * 128:(k + 1) * 128, :])
            nc.sync.dma_start(Qs[:], Q_e[:])
            nc.sync.dma_start(W2B[:], W2B_e[:])
            nc.sync.dma_start(B2V[:], B2V_e[:])
            nc.sync.dma_start(FC1W[:], FC1W_e[:])
            nc.sync.dma_start(FC1B[:], FC1B_e[:])
            nc.sync.dma_start(FC2W[:], FC2W_e[:])
            nc.sync.dma_start(FC2B[:], FC2B_e[:])
            nc.sync.dma_start(IDT[:], IDT_e[:])
            nc.sync.dma_start(CBs[:], CB_e[:])

            with tc.tile_pool(name="gps", bufs=1, space="PSUM") as gps:
                # ---- G1: conv1-dense matmuls -> c1d tiles (bf16) ----
                for t in range(10):
                    ncol = W1COLS[t]
                    off = int(w1off[t])
                    pg = gps.tile([128, BL], f32, tag="psg", name=f"psg{t}")
                    for k in range(2):
                        nc.tensor.matmul(
                            pg[0:ncol, :], W1[k][:, off:off + ncol], dTb[k][:],
                            start=(k == 0), stop=(k == 1))
                    nc.scalar.activation(c1d[t][0:ncol, :], pg[0:ncol, :],
                                         AF.Copy, bias=0.0, scale=1.0)

                # ---- G2: per-rule gating ----
                for r in range(R):
                    zpre = []
                    psz = gps.tile([128, BL], f32, tag="psz", name=f"psz{r}")
                    for jb in range(5):
                        kj, mj = KJB[jb], MJB[jb]
                        rl0 = wp.tile([128, BL], bf16, tag="rl0", name=f"rl0_{r}_{jb}")
                        rl1 = wp.tile([128, BL], bf16, tag="rl1", name=f"rl1_{r}_{jb}")
                        nc.vector.tensor_scalar(
                            rl0[0:kj, :], c1d[2 * jb][0:kj, :],
                            Qs[0:kj, r * 10 + 2 * jb:r * 10 + 2 * jb + 1],
                            0.0, ALU.add, ALU.max)
                        nc.vector.tensor_scalar(
                            rl1[0:kj, :], c1d[2 * jb + 1][0:kj, :],
                            Qs[0:kj, r * 10 + 2 * jb + 1:r * 10 + 2 * jb + 2],
                            0.0, ALU.add, ALU.max)
                        h1b = wp.tile([128, BL], bf16, tag="h1b", name=f"h1b_{r}_{jb}")
                        nc.vector.tensor_tensor(
                            h1b[0:kj, :], rl0[0:kj, :], rl1[0:kj, :], ALU.add)
                        ps2 = gps.tile([128, BL], f32, tag="ps2", name=f"ps2_{r}_{jb}")
                        nc.tensor.matmul(
                            ps2[0:mj, :],
                            W2B[0:kj, jb * 128:jb * 128 + mj],
                            h1b[0:kj, :], start=True, stop=True)
                        zp = wp.tile([128, BL], bf16, tag=f"zpre{jb}", name=f"zp_{r}_{jb}")
                        nc.scalar.activation(zp[0:mj, :], ps2[0:mj, :],
                                             AF.Relu, bias=B2V[0:mj, jb:jb + 1],
                                             scale=1.0)
                        zpre.append(zp)
                        nc.tensor.matmul(
                            psz[0:124, :],
                            FC1W[0:mj, jb * 124:(jb + 1) * 124],
                            zp[0:mj, :], start=(jb == 0), stop=(jb == 4))
                    zb = wp.tile([128, BL], bf16, tag="zb", name=f"zb{r}")
                    nc.scalar.activation(zb[0:124, :], psz[0:124, :],
                                         AF.Relu, bias=FC1B[0:124, :], scale=1.0)
                    psf = gps.tile([1, BL], f32, tag="psf", name=f"psf{r}")
                    nc.tensor.matmul(psf[0:1, :], FC2W[0:124, 0:1],
                                     zb[0:124, :], start=True, stop=True)
                    fst = wp.tile([1, BL], bf16, tag="fst", name=f"fst{r}")
                    nc.scalar.activation(fst[:], psf[0:1, :], AF.Tanh,
                                         bias=FC2B[0:1, :], scale=1.0)
                    nc.sync.dma_start(fsi[r:r + 1, :], fst[:])
                    nc.scalar.activation(eRows[0:1, r * BL:(r + 1) * BL],
                                         fst[:], AF.Exp, bias=0.0, scale=1.0)

                # ---- softmax pieces ----
                nc.scalar.activation(eTr[:], fsi[:], AF.Exp, bias=0.0, scale=1.0)
                for m in range(4):
                    pst = gps.tile([128, 32], bf16, tag="pst", name=f"pst{m}")
                    nc.tensor.transpose(pst[:], fsi[:, m * 128:(m + 1) * 128],
                                        IDT[:])
                    fse = wp.tile([128, 32], f32, tag="fse", name=f"fse{m}")
                    nc.scalar.activation(fse[:], pst[:], AF.Exp, bias=0.0,
                                         scale=1.0)
                    ssum = wp.tile([128, 1], f32, tag="ssum", name=f"ssum{m}")
                    nc.vector.reduce_sum(ssum[:], fse[:], AX.X)
                    nc.vector.reciprocal(recip[:, m:m + 1], ssum[:])

            # ---- expert phase ----
            with tc.tile_pool(name="eps", bufs=1, space="PSUM") as epp:
                eps = [epp.tile([128, NH], f32, tag=f"eps{g}", name=f"eps{g}") for g in range(8)]
                for r in range(R):
                    ebc = wp.tile([128, BL], f32, tag="ebc", name=f"ebc{r}")
                    nc.gpsimd.partition_broadcast(ebc[:], eRows[0:1, r * BL:(r + 1) * BL])
                    sd = []
                    for k in range(2):
                        sdk = wp.tile([128, BL], bf16, tag=f"sd{k}", name=f"sd_{r}_{k}")
                        nc.vector.tensor_tensor(sdk[:], dT[k][:], ebc[:],
                                                ALU.mult)
                        sd.append(sdk)
                    for k in range(2):
                        wt = wp.tile([128, NCLS], bf16, tag="wt", name=f"wt_{r}_{k}")
                        nc.sync.dma_start(
                            wt[:], CW_e[r, k * 128:(k + 1) * 128, :])
                        for m in range(4):
                            for n in range(2):
                                nc.tensor.matmul(
                                    eps[n * 4 + m][:],
                                    sd[k][:, m * 128:(m + 1) * 128],
                                    wt[:, n * NH:(n + 1) * NH],
                                    start=(r == 0 and k == 0), stop=False)
                # bias matmul + drain
                for n in range(2):
                    for m in range(4):
                        g = n * 4 + m
                        nc.tensor.matmul(
                            eps[g][:], eTr[:, m * 128:(m + 1) * 128],
                            CBs[:, n * NH:(n + 1) * NH], start=False, stop=True)
                        osb = wp.tile([128, NH], f32, tag="osb", name=f"osb_{n}_{m}")
                        nc.scalar.activation(osb[:], eps[g][:], AF.Copy,
                                             bias=0.0, scale=recip[:, m:m + 1])
                        nc.sync.dma_start(
                            OUT_e[m * 128:(m + 1) * 128, n * NH:(n + 1) * NH],
                            osb[:])
    nc.compile()
    return nc


_CACHE = {}


def kernel(data, proto, conv1_w, conv1_b, conv2_w, conv2_b,
           fc1_w, fc1_b, fc2_w, fc2_b, consq_w, consq_b, is_train=0,
           trace=False, tmpdir=None):
    bf = ml_dtypes.bfloat16
    data = np.asarray(data, np.float32)
    (W1, Q, W2B, B2V, FC1W, FC1B, FC2W, IDT) = _build_host(
        np.asarray(proto, np.float32), np.asarray(conv1_w, np.float32),
        np.asarray(conv1_b, np.float32), np.asarray(conv2_w, np.float32),
        np.asarray(conv2_b, np.float32), np.asarray(fc1_w, np.float32),
        np.asarray(fc1_b, np.float32), np.asarray(fc2_w, np.float32))
    if "nc" not in _CACHE:
        _CACHE["nc"] = _build_program()
    nc = _CACHE["nc"]

    CW = np.ascontiguousarray(np.asarray(consq_w, np.float32).astype(ml_dtypes.bfloat16))
    CB = np.ascontiguousarray(np.asarray(consq_b, np.float32).astype(ml_dtypes.bfloat16))
    FC2B = np.array([[np.asarray(fc2_b, np.float32).reshape(-1)[0]]], np.float32)
    shared = dict(W1=np.ascontiguousarray(W1), Q=np.ascontiguousarray(Q),
                  W2B=np.ascontiguousarray(W2B), B2V=np.ascontiguousarray(B2V),
                  FC1W=np.ascontiguousarray(FC1W),
                  FC1B=np.ascontiguousarray(FC1B),
                  FC2W=np.ascontiguousarray(FC2W), FC2B=FC2B,
                  IDT=np.ascontiguousarray(IDT), CB=CB, CW=CW)
    in_maps = []
    for i in range(NCORE):
        dsl = data[i * BL:(i + 1) * BL, :]
        dTi = np.ascontiguousarray(dsl.T)
        in_maps.append(dict(shared, dT=dTi,
                            dTb=np.ascontiguousarray(dTi.astype(bf))))
    res = run_bass_kernel_spmd(
        nc, in_maps, list(range(NCORE)), trace=trace,
        tmpdir=tmpdir or (tempfile.mkdtemp(prefix="moek_") if trace else None))
    out = np.concatenate([res.results[i]["OUT"] for i in range(NCORE)], axis=0)
    kernel.last_exec_time_ns = res.exec_time_ns
    return out


# revision 6
# speedup vs baseline: 2.1179x; 1.6038x over previous
import os
import sys
import tempfile

sys.path.insert(0, "/opt/trn_rl_repo")

import numpy as np
import ml_dtypes

import concourse.bacc as bacc
import concourse.mybir as mybir
import concourse.tile as tile
from concourse.bass_utils import run_bass_kernel_spmd

f32 = mybir.dt.float32
f32r = mybir.dt.float32r
bf16 = mybir.dt.bfloat16
AF = mybir.ActivationFunctionType
ALU = mybir.AluOpType
AX = mybir.AxisListType

# Problem dims (hardcoded per contract)
R, B, F, C, NCLS = 32, 4096, 256, 4, 1000
KK, PAD = 5, 1
L0, L1 = 254, 127          # conv1 out, pool1 out
J2 = 62                    # pool2 out positions
NCORE = 8
BL = B // NCORE            # 512 batch per core
NH = NCLS // 2             # 500, free-dim tile of expert matmul

# conv2 j2-blocks
SZ = [13, 13, 13, 13, 10]
JB0 = [0, 13, 26, 39, 52]                    # j2 block starts
BAND = []                                    # l1 band per block
for jb in range(5):
    lo = max(0, 26 * jb - 1)
    hi = min(126, 26 * jb + 2 * SZ[jb] + 2)
    BAND.append((lo, hi - lo + 1))
KJB = [4 * n for _, n in BAND]               # [116,120,120,120,96]
MJB = [8 * s for s in SZ]                    # [104,104,104,104,80]
W1COLS = [4 * n for _, n in BAND for _ in (0, 1)]  # per (jb,e) tile


def _conv1_np(x, w):
    # x: [N, F], w: [C,1,KK] -> [N, C, L0] with pad=1
    xp = np.pad(x, ((0, 0), (PAD, PAD)))
    out = np.zeros((x.shape[0], C, L0), np.float32)
    for c in range(C):
        for k in range(KK):
            out[:, c, :] += w[c, 0, k] * xp[:, k:k + L0]
    return out


def _build_host(proto, c1w, c1b, c2w, c2b, fc1w, fc1b, fc2w):
    bf = ml_dtypes.bfloat16
    # W1: dense conv1 matrix [F, sum(W1COLS)] in (jb,e) tile column order,
    # within tile col = l1loc*4 + c, conv output position (c, l0=2*l1+e)
    tot = sum(W1COLS)
    W1 = np.zeros((F, tot), np.float32)
    off = 0
    colmeta = []  # (jb, e, band_start, ncols)
    for jb in range(5):
        b0, bl = BAND[jb]
        for e in (0, 1):
            for l1loc in range(bl):
                l0 = 2 * (b0 + l1loc) + e
                for c in range(C):
                    col = off + l1loc * 4 + c
                    for k in range(KK):
                        f = l0 + k - 1
                        if 0 <= f < F:
                            W1[f, col] += c1w[c, 0, k]
            colmeta.append((jb, e, b0, 4 * bl))
            off += 4 * bl
    # Q: per-partition scalars [128, R*10] f32; col = r*10 + (jb*2+e)
    c1p = _conv1_np(proto, c1w)  # [R, C, L0]
    Q = np.zeros((128, R * 10), np.float32)
    for r in range(R):
        t = 0
        for jb in range(5):
            b0, bl = BAND[jb]
            for e in (0, 1):
                for l1loc in range(bl):
                    l0 = 2 * (b0 + l1loc) + e
                    for c in range(C):
                        Q[l1loc * 4 + c, r * 10 + t] = c1b[c] - c1p[r, c, l0]
                t += 1
    # W2B: banded conv2 [128, 5*128] bf16; block jb at free offset jb*128,
    # rows (l1loc, ci), cols (e2, j2loc, co); includes 0.5 pool1 scale
    W2B = np.zeros((128, 5 * 128), np.float32)
    for jb in range(5):
        b0, bl = BAND[jb]
        for e2 in (0, 1):
            for j2loc in range(SZ[jb]):
                l2 = 26 * jb + 2 * j2loc + e2
                for co in range(C):
                    col = e2 * 4 * SZ[jb] + j2loc * 4 + co
                    for kk in range(KK):
                        l1 = l2 - 1 + kk
                        if b0 <= l1 < b0 + bl:
                            for ci in range(C):
                                W2B[(l1 - b0) * 4 + ci, jb * 128 + col] += (
                                    0.5 * c2w[co, ci, kk])
    # B2V: relu2 bias [128, 5]
    B2V = np.zeros((128, 5), np.float32)
    for jb in range(5):
        for e2 in (0, 1):
            for j2loc in range(SZ[jb]):
                for co in range(C):
                    B2V[e2 * 4 * SZ[jb] + j2loc * 4 + co, jb] = c2b[co]
    # FC1W: [128, 5*124] bf16; block jb rows (e2,j2loc,co) -> 0.5*fc1w[co*62+j2]
    FC1W = np.zeros((128, 5 * 124), np.float32)
    for jb in range(5):
        for e2 in (0, 1):
            for j2loc in range(SZ[jb]):
                j2 = JB0[jb] + j2loc
                for co in range(C):
                    FC1W[e2 * 4 * SZ[jb] + j2loc * 4 + co,
                         jb * 124:(jb + 1) * 124] = 0.5 * fc1w[co * 62 + j2, :]
    FC1B = np.zeros((128, 1), np.float32)
    FC1B[:124, 0] = fc1b
    FC2W = np.zeros((128, 1), np.float32)
    FC2W[:124, 0] = fc2w[:, 0]
    IDT = np.eye(32, dtype=bf)
    return (W1.astype(bf), Q, W2B.astype(bf), B2V, FC1W.astype(bf), FC1B,
            FC2W.astype(bf), IDT)


def _build_program():
    nc = bacc.Bacc("TRN2", target_bir_lowering=False, debug=False,
                   num_devices=NCORE)
    TOT1 = sum(W1COLS)
    dT_e = nc.declare_dram_parameter("dT", [F, BL], f32, isOutput=False)
    dTb_e = nc.declare_dram_parameter("dTb", [F, BL], bf16, isOutput=False)
    W1_e = nc.declare_dram_parameter("W1", [F, TOT1], bf16, isOutput=False)
    Q_e = nc.declare_dram_parameter("Q", [128, R * 10], f32, isOutput=False)
    W2B_e = nc.declare_dram_parameter("W2B", [128, 5 * 128], bf16, isOutput=False)
    B2V_e = nc.declare_dram_parameter("B2V", [128, 5], f32, isOutput=False)
    FC1W_e = nc.declare_dram_parameter("FC1W", [128, 5 * 124], bf16, isOutput=False)
    FC1B_e = nc.declare_dram_parameter("FC1B", [128, 1], f32, isOutput=False)
    FC2W_e = nc.declare_dram_parameter("FC2W", [128, 1], bf16, isOutput=False)
    FC2B_e = nc.declare_dram_parameter("FC2B", [1, 1], f32, isOutput=False)
    IDT_e = nc.declare_dram_parameter("IDT", [32, 32], bf16, isOutput=False)
    CB_e = nc.declare_dram_parameter("CB", [R, NCLS], bf16, isOutput=False)
    CW_e = nc.declare_dram_parameter("CW", [R, F, NCLS], bf16, isOutput=False)
    OUT_e = nc.declare_dram_parameter("OUT", [BL, NCLS], f32, isOutput=True)

    # tile column offsets of W1 per (jb,e)
    w1off = np.cumsum([0] + W1COLS[:-1])

    with tile.TileContext(nc) as tc:
        with (
            tc.tile_pool(name="const", bufs=1) as cp,
            tc.tile_pool(name="work", bufs=3) as wp,
        ):
            dT = [cp.tile([128, BL], f32, tag=f"dT{k}", name=f"dT{k}") for k in range(2)]
            dTb = [cp.tile([128, BL], bf16, tag=f"dTb{k}", name=f"dTb{k}") for k in range(2)]
            W1 = [cp.tile([128, TOT1], bf16, tag=f"W1{k}", name=f"W1t{k}") for k in range(2)]
            Qs = cp.tile([128, R * 10], f32, tag="Qs")
            W2B = cp.tile([128, 5 * 128], bf16, tag="W2B")
            B2V = cp.tile([128, 5], f32, tag="B2V")
            FC1W = cp.tile([128, 5 * 124], bf16, tag="FC1W")
            FC1B = cp.tile([128, 1], f32, tag="FC1B")
            FC2W = cp.tile([128, 1], bf16, tag="FC2W")
            FC2B = cp.tile([1, 1], f32, tag="FC2B")
            IDT = cp.tile([32, 32], bf16, tag="IDT")
            CBs = cp.tile([R, NCLS], bf16, tag="CBs")
            fsi = cp.tile([R, BL], bf16, tag="fsi")
            eRows = cp.tile([1, R * BL], f32, tag="eRows")
            eTr = cp.tile([R, BL], bf16, tag="eTr")
            recip = cp.tile([128, 4], f32, tag="recip")
            c1d = [cp.tile([128, BL], bf16, tag=f"c1d{t}", name=f"c1d{t}") for t in range(10)]

            for k in range(2):
                nc.sync.dma_start(dT[k][:], dT_e[k * 128:(k + 1) * 128, :])
                nc.sync.dma_start(dTb[k][:], dTb_e[k * 128:(k + 1) * 128, :])
                nc.sync.dma_start(W1[k][:], W1_e[k * 128:(k + 1) * 128, :])
            nc.sync.dma_start(Qs[:], Q_e[:])
            nc.sync.dma_start(W2B[:], W2B_e[:])
            nc.sync.dma_start(B2V[:], B2V_e[:])
            nc.sync.dma_start(FC1W[:], FC1W_e[:])
            nc.sync.dma_start(FC1B[:], FC1B_e[:])
            nc.sync.dma_start(FC2W[:], FC2W_e[:])
            nc.sync.dma_start(FC2B[:], FC2B_e[:])
            nc.sync.dma_start(IDT[:], IDT_e[:])
            nc.sync.dma_start(CBs[:], CB_e[:])

            with tc.tile_pool(name="gps", bufs=1, space="PSUM") as gps:
                # ---- G1: conv1-dense matmuls -> c1d tiles (bf16) ----
                for t in range(10):
                    ncol = W1COLS[t]
                    off = int(w1off[t])
                    pg = gps.tile([128, BL], f32, tag="psg", name=f"psg{t}")
                    for k in range(2):
                        nc.tensor.matmul(
                            pg[0:ncol, :], W1[k][:, off:off + ncol], dTb[k][:],
                            start=(k == 0), stop=(k == 1))
                    nc.scalar.activation(c1d[t][0:ncol, :], pg[0:ncol, :],
                                         AF.Copy, bias=0.0, scale=1.0)

                # ---- G2: per-rule gating ----
                for r in range(R):
                    zpre = []
                    psz = gps.tile([128, BL], f32, tag="psz", name=f"psz{r}")
                    for jb in range(5):
                        kj, mj = KJB[jb], MJB[jb]
                        rl0 = wp.tile([128, BL], bf16, tag="rl0", name=f"rl0_{r}_{jb}")
                        rl1 = wp.tile([128, BL], bf16, tag="rl1", name=f"rl1_{r}_{jb}")
                        nc.vector.tensor_scalar(
                            rl0[0:kj, :], c1d[2 * jb][0:kj, :],
                            Qs[0:kj, r * 10 + 2 * jb:r * 10 + 2 * jb + 1],
                            0.0, ALU.add, ALU.max)
                        nc.vector.tensor_scalar(
                            rl1[0:kj, :], c1d[2 * jb + 1][0:kj, :],
                            Qs[0:kj, r * 10 + 2 * jb + 1:r * 10 + 2 * jb + 2],
                            0.0, ALU.add, ALU.max)
                        h1b = wp.tile([128, BL], bf16, tag="h1b", name=f"h1b_{r}_{jb}")
                        nc.vector.tensor_tensor(
                            h1b[0:kj, :], rl0[0:kj, :], rl1[0:kj, :], ALU.add)
                        ps2 = gps.tile([128, BL], f32, tag="ps2", name=f"ps2_{r}_{jb}")
                        nc.tensor.matmul(
                            ps2[0:mj, :],
                            W2B[0:kj, jb * 128:jb * 128 + mj],
                            h1b[0:kj, :], start=True, stop=True)
                        zp = wp.tile([128, BL], bf16, tag=f"zpre{jb}", name=f"zp_{r}_{jb}")
                        nc.scalar.activation(zp[0:mj, :], ps2[0:mj, :],
                                             AF.Relu, bias=B2V[0:mj, jb:jb + 1],
                                             scale=1.0)
                        zpre.append(zp)
                        nc.tensor.matmul(
                            psz[0:124, :],
                            FC1W[0:mj, jb * 124:(jb + 1) * 124],
                            zp[0:mj, :], start=(jb == 0), stop=(jb == 4))
                    zb = wp.tile([128, BL], bf16, tag="zb", name=f"zb{r}")
                    nc.scalar.activation(zb[0:124, :], psz[0:124, :],
                                         AF.Relu, bias=FC1B[0:124, :], scale=1.0)
                    psf = gps.tile([1, BL], f32, tag="psf", name=f"psf{r}")
                    nc.tensor.matmul(psf[0:1, :], FC2W[0:124, 0:1],
                                     zb[0:124, :], start=True, stop=True)
                    fst = wp.tile([1, BL], bf16, tag="fst", name=f"fst{r}")
                    nc.scalar.activation(fst[:], psf[0:1, :], AF.Tanh,
                                         bias=FC2B[0:1, :], scale=1.0)
                    nc.sync.dma_start(fsi[r:r + 1, :], fst[:])
                    nc.scalar.activation(eRows[0:1, r * BL:(r + 1) * BL],
                                         fst[:], AF.Exp, bias=0.0, scale=1.0)

                # ---- softmax pieces ----
                nc.scalar.activation(eTr[:], fsi[:], AF.Exp, bias=0.0, scale=1.0)
                for m in range(4):
                    pst = gps.tile([128, 32], bf16, tag="pst", name=f"pst{m}")
                    nc.tensor.transpose(pst[:], fsi[:, m * 128:(m + 1) * 128],
                                        IDT[:])
                    fse = wp.tile([128, 32], f32, tag="fse", name=f"fse{m}")
                    nc.scalar.activation(fse[:], pst[:], AF.Exp, bias=0.0,
                                         scale=1.0)
                    ssum = wp.tile([128, 1], f32, tag="ssum", name=f"ssum{m}")
                    nc.vector.reduce_sum(ssum[:], fse[:], AX.X)
                    nc.vector.reciprocal(recip[:, m:m + 1], ssum[:])

            # ---- expert phase ----
            with tc.tile_pool(name="eps", bufs=1, space="PSUM") as epp:
                eps = [epp.tile([128, NH], f32, tag=f"eps{g}", name=f"eps{g}") for g in range(8)]
                for r in range(R):
                    ebc = wp.tile([128, BL], f32, tag="ebc", name=f"ebc{r}")
                    nc.gpsimd.partition_broadcast(ebc[:], eRows[0:1, r * BL:(r + 1) * BL])
                    sd = []
                    for k in range(2):
                        sdk = wp.tile([128, BL], bf16, tag=f"sd{k}", name=f"sd_{r}_{k}")
                        nc.vector.tensor_tensor(sdk[:], dT[k][:], ebc[:],
                                                ALU.mult)
                        sd.append(sdk)
                    for k in range(2):
                        wt = wp.tile([128, NCLS], bf16, tag="wt", name=f"wt_{r}_{k}")
                        nc.sync.dma_start(
                            wt[:], CW_e[r, k * 128:(k + 1) * 128, :])
                        for m in range(4):
                            for n in range(2):
                                nc.tensor.matmul(
                                    eps[n * 4 + m][:],
                                    sd[k][:, m * 128:(m + 1) * 128],
                                    wt[:, n * NH:(n + 1) * NH],
                                    start=(r == 0 and k == 0), stop=False)
                # bias matmul + drain
                for n in range(2):
                    for m in range(4):
                        g = n * 4 + m
                        nc.tensor.matmul(
                            eps[g][:], eTr[:, m * 128:(m + 1) * 128],
                            CBs[:, n * NH:(n + 1) * NH], start=False, stop=True)
                        osb = wp.tile([128, NH], f32, tag="osb", name=f"osb_{n}_{m}")
                        nc.scalar.activation(osb[:], eps[g][:], AF.Copy,
                                             bias=0.0, scale=recip[:, m:m + 1])
                        nc.sync.dma_start(
                            OUT_e[m * 128:(m + 1) * 128, n * NH:(n + 1) * NH],
                            osb[:])
    nc.compile()
    return nc


_CACHE = {}


def kernel(data, proto, conv1_w, conv1_b, conv2_w, conv2_b,
           fc1_w, fc1_b, fc2_w, fc2_b, consq_w, consq_b, is_train=0,
           trace=False, tmpdir=None):
    bf = ml_dtypes.bfloat16
    data = np.asarray(data, np.float32)
    (W1, Q, W2B, B2V, FC1W, FC1B, FC2W, IDT) = _build_host(
        np.asarray(proto, np.float32), np.asarray(conv1_w, np.float32),
        np.asarray(conv1_b, np.float32), np.asarray(conv2_w, np.float32),
        np.asarray(conv2_b, np.float32), np.asarray(fc1_w, np.float32),
        np.asarray(fc1_b, np.float32), np.asarray(fc2_w, np.float32))
    if "nc" not in _CACHE:
        _CACHE["nc"] = _build_program()
    nc = _CACHE["nc"]

    CW = np.ascontiguousarray(np.asarray(consq_w, np.float32).astype(ml_dtypes.bfloat16))
    CB = np.ascontiguousarray(np.asarray(consq_b, np.float32).astype(ml_dtypes.bfloat16))
    FC2B = np.array([[np.asarray(fc2_b, np.float32).reshape(-1)[0]]], np.float32)
    shared = dict(W1=np.ascontiguousarray(W1), Q=np.ascontiguousarray(Q),
                  W2B=np.ascontiguousarray(W2B), B2V=np.ascontiguousarray(B2V),
                  FC1W=np.ascontiguousarray(FC1W),
                  FC1B=np.ascontiguousarray(FC1B),
                  FC2W=np.ascontiguousarray(FC2W), FC2B=FC2B,
                  IDT=np.ascontiguousarray(IDT), CB=CB, CW=CW)
    in_maps = []
    for i in range(NCORE):
        dsl = data[i * BL:(i + 1) * BL, :]
        dTi = np.ascontiguousarray(dsl.T)
        in_maps.append(dict(shared, dT=dTi,
                            dTb=np.ascontiguousarray(dTi.astype(bf))))
    res = run_bass_kernel_spmd(
        nc, in_maps, list(range(NCORE)), trace=trace,
        tmpdir=tmpdir or (tempfile.mkdtemp(prefix="moek_") if trace else None))
    out = np.concatenate([res.results[i]["OUT"] for i in range(NCORE)], axis=0)
    kernel.last_exec_time_ns = res.exec_time_ns
    return out


# revision 7
# speedup vs baseline: 2.6302x; 1.2419x over previous
import os
import sys
import tempfile

sys.path.insert(0, "/opt/trn_rl_repo")

import numpy as np
import ml_dtypes

import concourse.bacc as bacc
import concourse.mybir as mybir
import concourse.tile as tile
from concourse.bass_utils import run_bass_kernel_spmd

f32 = mybir.dt.float32
f32r = mybir.dt.float32r
bf16 = mybir.dt.bfloat16
AF = mybir.ActivationFunctionType
ALU = mybir.AluOpType
AX = mybir.AxisListType

# Problem dims (hardcoded per contract)
R, B, F, C, NCLS = 32, 4096, 256, 4, 1000
KK, PAD = 5, 1
L0, L1 = 254, 127          # conv1 out, pool1 out
J2 = 62                    # pool2 out positions
NCORE = 8
BL = B // NCORE            # 512 batch per core
NH = NCLS // 2             # 500, free-dim tile of expert matmul

# conv2 j2-blocks
SZ = [13, 13, 13, 13, 10]
JB0 = [0, 13, 26, 39, 52]                    # j2 block starts
BAND = []                                    # l1 band per block
for jb in range(5):
    lo = max(0, 26 * jb - 1)
    hi = min(126, 26 * jb + 2 * SZ[jb] + 2)
    BAND.append((lo, hi - lo + 1))
KJB = [4 * n for _, n in BAND]               # [116,120,120,120,96]
MJB = [8 * s for s in SZ]                    # [104,104,104,104,80]
W1COLS = [4 * n for _, n in BAND for _ in (0, 1)]  # per (jb,e) tile


def _conv1_np(x, w):
    # x: [N, F], w: [C,1,KK] -> [N, C, L0] with pad=1
    xp = np.pad(x, ((0, 0), (PAD, PAD)))
    out = np.zeros((x.shape[0], C, L0), np.float32)
    for c in range(C):
        for k in range(KK):
            out[:, c, :] += w[c, 0, k] * xp[:, k:k + L0]
    return out


def _build_host(proto, c1w, c1b, c2w, c2b, fc1w, fc1b, fc2w):
    bf = ml_dtypes.bfloat16
    # W1: dense conv1 matrix [F, sum(W1COLS)] in (jb,e) tile column order,
    # within tile col = l1loc*4 + c, conv output position (c, l0=2*l1+e)
    tot = sum(W1COLS)
    W1 = np.zeros((F, tot), np.float32)
    off = 0
    colmeta = []  # (jb, e, band_start, ncols)
    for jb in range(5):
        b0, bl = BAND[jb]
        for e in (0, 1):
            for l1loc in range(bl):
                l0 = 2 * (b0 + l1loc) + e
                for c in range(C):
                    col = off + l1loc * 4 + c
                    for k in range(KK):
                        f = l0 + k - 1
                        if 0 <= f < F:
                            W1[f, col] += c1w[c, 0, k]
            colmeta.append((jb, e, b0, 4 * bl))
            off += 4 * bl
    # Q: per-partition scalars [128, R*10] f32; col = r*10 + (jb*2+e)
    c1p = _conv1_np(proto, c1w)  # [R, C, L0]
    Q = np.zeros((128, R * 10), np.float32)
    for r in range(R):
        t = 0
        for jb in range(5):
            b0, bl = BAND[jb]
            for e in (0, 1):
                for l1loc in range(bl):
                    l0 = 2 * (b0 + l1loc) + e
                    for c in range(C):
                        Q[l1loc * 4 + c, r * 10 + t] = c1b[c] - c1p[r, c, l0]
                t += 1
    # W2B: banded conv2 [128, 5*128] bf16; block jb at free offset jb*128,
    # rows (l1loc, ci), cols (e2, j2loc, co); includes 0.5 pool1 scale
    W2B = np.zeros((128, 5 * 128), np.float32)
    for jb in range(5):
        b0, bl = BAND[jb]
        for e2 in (0, 1):
            for j2loc in range(SZ[jb]):
                l2 = 26 * jb + 2 * j2loc + e2
                for co in range(C):
                    col = e2 * 4 * SZ[jb] + j2loc * 4 + co
                    for kk in range(KK):
                        l1 = l2 - 1 + kk
                        if b0 <= l1 < b0 + bl:
                            for ci in range(C):
                                W2B[(l1 - b0) * 4 + ci, jb * 128 + col] += (
                                    0.5 * c2w[co, ci, kk])
    # B2V: relu2 bias [128, 5]
    B2V = np.zeros((128, 5), np.float32)
    for jb in range(5):
        for e2 in (0, 1):
            for j2loc in range(SZ[jb]):
                for co in range(C):
                    B2V[e2 * 4 * SZ[jb] + j2loc * 4 + co, jb] = c2b[co]
    # FC1W: [128, 5*124] bf16; block jb rows (e2,j2loc,co) -> 0.5*fc1w[co*62+j2]
    FC1W = np.zeros((128, 5 * 124), np.float32)
    for jb in range(5):
        for e2 in (0, 1):
            for j2loc in range(SZ[jb]):
                j2 = JB0[jb] + j2loc
                for co in range(C):
                    FC1W[e2 * 4 * SZ[jb] + j2loc * 4 + co,
                         jb * 124:(jb + 1) * 124] = 0.5 * fc1w[co * 62 + j2, :]
    FC1B = np.zeros((128, 1), np.float32)
    FC1B[:124, 0] = fc1b
    FC2W = np.zeros((128, 1), np.float32)
    FC2W[:124, 0] = fc2w[:, 0]
    IDT = np.eye(32, dtype=bf)
    return (W1.astype(bf), Q, W2B.astype(bf), B2V, FC1W.astype(bf), FC1B,
            FC2W.astype(bf), IDT)


def _build_program():
    nc = bacc.Bacc("TRN2", target_bir_lowering=False, debug=False,
                   num_devices=NCORE)
    TOT1 = sum(W1COLS)
    dT_e = nc.declare_dram_parameter("dT", [F, BL], f32, isOutput=False)
    dTb_e = nc.declare_dram_parameter("dTb", [F, BL], bf16, isOutput=False)
    W1_e = nc.declare_dram_parameter("W1", [F, TOT1], bf16, isOutput=False)
    Q_e = nc.declare_dram_parameter("Q", [128, R * 10], f32, isOutput=False)
    W2B_e = nc.declare_dram_parameter("W2B", [128, 5 * 128], bf16, isOutput=False)
    B2V_e = nc.declare_dram_parameter("B2V", [128, 5], f32, isOutput=False)
    FC1W_e = nc.declare_dram_parameter("FC1W", [128, 5 * 124], bf16, isOutput=False)
    FC1B_e = nc.declare_dram_parameter("FC1B", [128, 1], f32, isOutput=False)
    FC2W_e = nc.declare_dram_parameter("FC2W", [128, 1], bf16, isOutput=False)
    FC2B_e = nc.declare_dram_parameter("FC2B", [1, 1], f32, isOutput=False)
    IDT_e = nc.declare_dram_parameter("IDT", [32, 32], bf16, isOutput=False)
    CB_e = nc.declare_dram_parameter("CB", [R, NCLS], bf16, isOutput=False)
    CW_e = nc.declare_dram_parameter("CW", [R, F, NCLS], bf16, isOutput=False)
    OUT_e = nc.declare_dram_parameter("OUT", [BL, NCLS], f32, isOutput=True)

    # tile column offsets of W1 per (jb,e)
    w1off = np.cumsum([0] + W1COLS[:-1])

    with tile.TileContext(nc) as tc:
        with (
            tc.tile_pool(name="const", bufs=1) as cp,
            tc.tile_pool(name="work", bufs=3) as wp,
        ):
            dT = [cp.tile([128, BL], f32, tag=f"dT{k}", name=f"dT{k}") for k in range(2)]
            dTb = [cp.tile([128, BL], bf16, tag=f"dTb{k}", name=f"dTb{k}") for k in range(2)]
            W1 = [cp.tile([128, TOT1], bf16, tag=f"W1{k}", name=f"W1t{k}") for k in range(2)]
            Qs = cp.tile([128, R * 10], f32, tag="Qs")
            W2B = cp.tile([128, 5 * 128], bf16, tag="W2B")
            B2V = cp.tile([128, 5], f32, tag="B2V")
            FC1W = cp.tile([128, 5 * 124], bf16, tag="FC1W")
            FC1B = cp.tile([128, 1], f32, tag="FC1B")
            FC2W = cp.tile([128, 1], bf16, tag="FC2W")
            FC2B = cp.tile([1, 1], f32, tag="FC2B")
            IDT = cp.tile([32, 32], bf16, tag="IDT")
            CBs = cp.tile([R, NCLS], bf16, tag="CBs")
            fsi = cp.tile([R, BL], bf16, tag="fsi")
            eRows = cp.tile([1, R * BL], f32, tag="eRows")
            eTr = cp.tile([R, BL], bf16, tag="eTr")
            recip = cp.tile([128, 4], f32, tag="recip")
            c1d = [cp.tile([128, BL], bf16, tag=f"c1d{t}", name=f"c1d{t}") for t in range(10)]

            for k in range(2):
                nc.sync.dma_start(dT[k][:], dT_e[k * 128:(k + 1) * 128, :])
                nc.sync.dma_start(dTb[k][:], dTb_e[k * 128:(k + 1) * 128, :])
                nc.sync.dma_start(W1[k][:], W1_e[k * 128:(k + 1) * 128, :])
            nc.sync.dma_start(Qs[:], Q_e[:])
            nc.sync.dma_start(W2B[:], W2B_e[:])
            nc.sync.dma_start(B2V[:], B2V_e[:])
            nc.sync.dma_start(FC1W[:], FC1W_e[:])
            nc.sync.dma_start(FC1B[:], FC1B_e[:])
            nc.sync.dma_start(FC2W[:], FC2W_e[:])
            nc.sync.dma_start(FC2B[:], FC2B_e[:])
            nc.sync.dma_start(IDT[:], IDT_e[:])
            nc.sync.dma_start(CBs[:], CB_e[:])

            with tc.tile_pool(name="gps", bufs=1, space="PSUM") as gps:
                # ---- G1: conv1-dense matmuls -> c1d tiles (bf16) ----
                for t in range(10):
                    ncol = W1COLS[t]
                    off = int(w1off[t])
                    pg = gps.tile([128, BL], f32, tag="psg", name=f"psg{t}")
                    for k in range(2):
                        nc.tensor.matmul(
                            pg[0:ncol, :], W1[k][:, off:off + ncol], dTb[k][:],
                            start=(k == 0), stop=(k == 1))
                    nc.scalar.activation(c1d[t][0:ncol, :], pg[0:ncol, :],
                                         AF.Copy, bias=0.0, scale=1.0)

                # ---- G2: per-rule gating ----
                for r in range(R):
                    zpre = []
                    psz = gps.tile([128, BL], f32, tag=f"psz{r % 2}", name=f"psz{r}")
                    for jb in range(5):
                        kj, mj = KJB[jb], MJB[jb]
                        rl0 = wp.tile([128, BL], bf16, tag="rl0", name=f"rl0_{r}_{jb}")
                        rl1 = wp.tile([128, BL], bf16, tag="rl1", name=f"rl1_{r}_{jb}")
                        nc.vector.tensor_scalar(
                            rl0[0:kj, :], c1d[2 * jb][0:kj, :],
                            Qs[0:kj, r * 10 + 2 * jb:r * 10 + 2 * jb + 1],
                            0.0, ALU.add, ALU.max)
                        nc.vector.tensor_scalar(
                            rl1[0:kj, :], c1d[2 * jb + 1][0:kj, :],
                            Qs[0:kj, r * 10 + 2 * jb + 1:r * 10 + 2 * jb + 2],
                            0.0, ALU.add, ALU.max)
                        h1b = wp.tile([128, BL], bf16, tag="h1b", name=f"h1b_{r}_{jb}")
                        nc.vector.tensor_tensor(
                            h1b[0:kj, :], rl0[0:kj, :], rl1[0:kj, :], ALU.add)
                        ps2 = gps.tile([128, BL], f32, tag=f"ps2_{r % 2}", name=f"ps2_{r}_{jb}")
                        nc.tensor.matmul(
                            ps2[0:mj, :],
                            W2B[0:kj, jb * 128:jb * 128 + mj],
                            h1b[0:kj, :], start=True, stop=True)
                        zp = wp.tile([128, BL], bf16, tag=f"zpre{jb}", name=f"zp_{r}_{jb}")
                        nc.scalar.activation(zp[0:mj, :], ps2[0:mj, :],
                                             AF.Relu, bias=B2V[0:mj, jb:jb + 1],
                                             scale=1.0)
                        zpre.append(zp)
                        nc.tensor.matmul(
                            psz[0:124, :],
                            FC1W[0:mj, jb * 124:(jb + 1) * 124],
                            zp[0:mj, :], start=(jb == 0), stop=(jb == 4))
                    zb = wp.tile([128, BL], bf16, tag="zb", name=f"zb{r}")
                    nc.scalar.activation(zb[0:124, :], psz[0:124, :],
                                         AF.Relu, bias=FC1B[0:124, :], scale=1.0)
                    psf = gps.tile([1, BL], f32, tag="psf", name=f"psf{r}")
                    nc.tensor.matmul(psf[0:1, :], FC2W[0:124, 0:1],
                                     zb[0:124, :], start=True, stop=True)
                    fst = wp.tile([1, BL], bf16, tag="fst", name=f"fst{r}")
                    nc.scalar.activation(fst[:], psf[0:1, :], AF.Tanh,
                                         bias=FC2B[0:1, :], scale=1.0)
                    nc.sync.dma_start(fsi[r:r + 1, :], fst[:])
                    nc.scalar.activation(eRows[0:1, r * BL:(r + 1) * BL],
                                         fst[:], AF.Exp, bias=0.0, scale=1.0)

                # ---- softmax pieces ----
                nc.scalar.activation(eTr[:], fsi[:], AF.Exp, bias=0.0, scale=1.0)
                for m in range(4):
                    pst = gps.tile([128, 32], bf16, tag="pst", name=f"pst{m}")
                    nc.tensor.transpose(pst[:], fsi[:, m * 128:(m + 1) * 128],
                                        IDT[:])
                    fse = wp.tile([128, 32], f32, tag="fse", name=f"fse{m}")
                    nc.scalar.activation(fse[:], pst[:], AF.Exp, bias=0.0,
                                         scale=1.0)
                    ssum = wp.tile([128, 1], f32, tag="ssum", name=f"ssum{m}")
                    nc.vector.reduce_sum(ssum[:], fse[:], AX.X)
                    nc.vector.reciprocal(recip[:, m:m + 1], ssum[:])

            # ---- expert phase ----
            with tc.tile_pool(name="eps", bufs=1, space="PSUM") as epp:
                eps = [epp.tile([128, NH], f32, tag=f"eps{g}", name=f"eps{g}") for g in range(8)]
                for r in range(R):
                    ebc = wp.tile([128, BL], f32, tag="ebc", name=f"ebc{r}")
                    nc.gpsimd.partition_broadcast(ebc[:], eRows[0:1, r * BL:(r + 1) * BL])
                    sd = []
                    for k in range(2):
                        sdk = wp.tile([128, BL], bf16, tag=f"sd{k}", name=f"sd_{r}_{k}")
                        nc.vector.tensor_tensor(sdk[:], dT[k][:], ebc[:],
                                                ALU.mult)
                        sd.append(sdk)
                    for k in range(2):
                        wt = wp.tile([128, NCLS], bf16, tag="wt", name=f"wt_{r}_{k}")
                        nc.sync.dma_start(
                            wt[:], CW_e[r, k * 128:(k + 1) * 128, :])
                        for m in range(4):
                            for n in range(2):
                                nc.tensor.matmul(
                                    eps[n * 4 + m][:],
                                    sd[k][:, m * 128:(m + 1) * 128],
                                    wt[:, n * NH:(n + 1) * NH],
                                    start=(r == 0 and k == 0), stop=False)
                # bias matmul + drain
                for n in range(2):
                    for m in range(4):
                        g = n * 4 + m
                        nc.tensor.matmul(
                            eps[g][:], eTr[:, m * 128:(m + 1) * 128],
                            CBs[:, n * NH:(n + 1) * NH], start=False, stop=True)
                        osb = wp.tile([128, NH], f32, tag="osb", name=f"osb_{n}_{m}")
                        nc.scalar.activation(osb[:], eps[g][:], AF.Copy,
                                             bias=0.0, scale=recip[:, m:m + 1])
                        nc.sync.dma_start(
                            OUT_e[m * 128:(m + 1) * 128, n * NH:(n + 1) * NH],
                            osb[:])
    nc.compile()
    return nc


_CACHE = {}


def kernel(data, proto, conv1_w, conv1_b, conv2_w, conv2_b,
           fc1_w, fc1_b, fc2_w, fc2_b, consq_w, consq_b, is_train=0,
           trace=False, tmpdir=None):
    bf = ml_dtypes.bfloat16
    data = np.asarray(data, np.float32)
    (W1, Q, W2B, B2V, FC1W, FC1B, FC2W, IDT) = _build_host(
        np.asarray(proto, np.float32), np.asarray(conv1_w, np.float32),
        np.asarray(conv1_b, np.float32), np.asarray(conv2_w, np.float32),
        np.asarray(conv2_b, np.float32), np.asarray(fc1_w, np.float32),
        np.asarray(fc1_b, np.float32), np.asarray(fc2_w, np.float32))
    if "nc" not in _CACHE:
        _CACHE["nc"] = _build_program()
    nc = _CACHE["nc"]

    CW = np.ascontiguousarray(np.asarray(consq_w, np.float32).astype(ml_dtypes.bfloat16))
    CB = np.ascontiguousarray(np.asarray(consq_b, np.float32).astype(ml_dtypes.bfloat16))
    FC2B = np.array([[np.asarray(fc2_b, np.float32).reshape(-1)[0]]], np.float32)
    shared = dict(W1=np.ascontiguousarray(W1), Q=np.ascontiguousarray(Q),
                  W2B=np.ascontiguousarray(W2B), B2V=np.ascontiguousarray(B2V),
                  FC1W=np.ascontiguousarray(FC1W),
                  FC1B=np.ascontiguousarray(FC1B),
                  FC2W=np.ascontiguousarray(FC2W), FC2B=FC2B,
                  IDT=np.ascontiguousarray(IDT), CB=CB, CW=CW)
    in_maps = []
    for i in range(NCORE):
        dsl = data[i * BL:(i + 1) * BL, :]
        dTi = np.ascontiguousarray(dsl.T)
        in_maps.append(dict(shared, dT=dTi,
                            dTb=np.ascontiguousarray(dTi.astype(bf))))
    res = run_bass_kernel_spmd(
        nc, in_maps, list(range(NCORE)), trace=trace,
        tmpdir=tmpdir or (tempfile.mkdtemp(prefix="moek_") if trace else None))
    out = np.concatenate([res.results[i]["OUT"] for i in range(NCORE)], axis=0)
    kernel.last_exec_time_ns = res.exec_time_ns
    return out
